# revision 1
# baseline (speedup 1.0000x reference)
"""Trainium2 Bass kernel for gumbel hard-attention (sparse_attention).

Math: out = one_hot(argmax_k((mask(q@k^T)*scale + gumbel)))) @ v @ w_proj + b_proj
Since hard gumbel-softmax forward value is exactly the one-hot row (up to 1ulp),
y[b,h,q,:] = v[b,h,argmax_k,:]; softmax is monotone so argmax over logits suffices.

Sharding: 24 (b,h) pairs over 8 cores -> 3 heads/core, one batch per core group
(cores 0-3: b=0, cores 4-7: b=1). Tensor-parallel c_proj partial sums reduced on host.
"""

import sys

for p in ("/opt/trn_rl_repo",):
    if p not in sys.path:
        sys.path.insert(0, p)

import numpy as np

import concourse.bacc as bacc
import concourse.bass as bass
import concourse.mybir as mybir
from concourse.bass_utils import run_bass_kernel_spmd
from concourse.tile import TileContext

F32 = mybir.dt.float32
U16 = mybir.dt.uint16
I16 = mybir.dt.int16

T = 2048          # sequence length
C = 768           # model dim
HPC = 3           # heads per core
HD = 64           # head dim
NB = T // 128     # 16 q-blocks
EC = C // 128     # 6 contraction chunks
NEG = -1e9


def build_program(debug=False, trace_sim=False):
    nc = bacc.Bacc(target_bir_lowering=False, trn_type="TRN2")

    x = nc.dram_tensor("x", [T, C], F32, kind="ExternalInput")
    wq = nc.dram_tensor("wq", [C, HPC * HD], F32, kind="ExternalInput")   # pre-scaled by 1/8
    wk = nc.dram_tensor("wk", [C, HPC * HD], F32, kind="ExternalInput")
    wv = nc.dram_tensor("wv", [C, HPC * HD], F32, kind="ExternalInput")
    wp = nc.dram_tensor("wp", [HPC * HD, C], F32, kind="ExternalInput")
    gum = nc.dram_tensor("gum", [HPC, T, T], F32, kind="ExternalInput")
    ident = nc.dram_tensor("ident", [128, 128], F32, kind="ExternalInput")
    mdiag = nc.dram_tensor("mdiag", [128, 128], F32, kind="ExternalInput")
    iotas = nc.dram_tensor("iotas", [128, NB], F32, kind="ExternalInput")
    onesel = nc.dram_tensor("onesel", [16, NB * 128], F32, kind="ExternalInput")

    out = nc.dram_tensor("out", [T, C], F32, kind="ExternalOutput")
    if debug:
        idx_dbg = nc.dram_tensor("idx_dbg", [HPC, 128, NB], F32, kind="ExternalOutput")
        yta_dbg = nc.dram_tensor("yta_dbg", [128, T], F32, kind="ExternalOutput")
        ytb_dbg = nc.dram_tensor("ytb_dbg", [64, T], F32, kind="ExternalOutput")


    with TileContext(nc, trace_sim=trace_sim) as tc:
        with (
            tc.tile_pool(name="const", bufs=1) as cpool,
            tc.tile_pool(name="big", bufs=1) as bigpool,
            tc.tile_pool(name="io", bufs=3) as iopool,
            tc.tile_pool(name="stat", bufs=4) as spool,
            tc.tile_pool(name="pstr", bufs=2, space="PSUM") as ptr,
            tc.tile_pool(name="psmm", bufs=2, space="PSUM") as pmm,
            tc.tile_pool(name="pso", bufs=1, space="PSUM") as psopool,
        ):
            id_sb = cpool.tile([128, 128], F32, tag="ident")
            nc.sync.dma_start(id_sb, ident[:, :])
            md_sb = cpool.tile([128, 128], F32, tag="mdiag")
            nc.sync.dma_start(md_sb, mdiag[:, :])
            io_sb = cpool.tile([128, NB], F32, tag="iotas")
            nc.sync.dma_start(io_sb, iotas[:, :])
            on_sb = cpool.tile([16, NB * 128], F32, tag="onesel")
            nc.sync.dma_start(on_sb, onesel[:, :])

            # ---- Phase 1: xT [128, EC, T] (e on partitions) via PE transposes
            xtpool = tc.tile_pool(name="xt", bufs=1)
            xtctx = xtpool.__enter__()
            xT = xtctx.tile([128, EC, T], F32, tag="xT")
            for tb in range(NB):
                xin = iopool.tile([128, C], F32, tag="xin")
                nc.sync.dma_start(xin, x[tb * 128:(tb + 1) * 128, :])
                for ec in range(EC):
                    pt = ptr.tile([128, 128], F32, tag="tr")
                    nc.tensor.transpose(pt, xin[:, ec * 128:(ec + 1) * 128], id_sb)
                    nc.scalar.copy(xT[:, ec, tb * 128:(tb + 1) * 128], pt)

            # ---- Phase 2: load weights, build Qt/Kt/Vt transposed pieces
            # piece A: cols 0:128 (heads 0,1), piece B: cols 128:192 (head 2)
            wq_sb = bigpool.tile([128, EC, HPC * HD], F32, tag="wq")
            wk_sb = bigpool.tile([128, EC, HPC * HD], F32, tag="wk")
            wv_sb = bigpool.tile([128, EC, HPC * HD], F32, tag="wv")
            for ec in range(EC):
                nc.sync.dma_start(wq_sb[:, ec, :], wq[ec * 128:(ec + 1) * 128, :])
                nc.sync.dma_start(wk_sb[:, ec, :], wk[ec * 128:(ec + 1) * 128, :])
                nc.sync.dma_start(wv_sb[:, ec, :], wv[ec * 128:(ec + 1) * 128, :])

            qtA = bigpool.tile([128, T], F32, tag="qtA")
            qtB = bigpool.tile([64, T], F32, tag="qtB")
            ktA = bigpool.tile([128, T], F32, tag="ktA")
            ktB = bigpool.tile([64, T], F32, tag="ktB")
            vtA = bigpool.tile([128, T], F32, tag="vtA")
            vtB = bigpool.tile([64, T], F32, tag="vtB")

            pieces = [
                (qtA, wq_sb, 0, 128), (qtB, wq_sb, 128, 64),
                (ktA, wk_sb, 0, 128), (ktB, wk_sb, 128, 64),
                (vtA, wv_sb, 0, 128), (vtB, wv_sb, 128, 64),
            ]
            for dst, wsb, c0, cn in pieces:
                for t4 in range(T // 512):
                    pp = pmm.tile([128, 512], F32, tag="mm")
                    for ec in range(EC):
                        nc.tensor.matmul(
                            pp[:cn, :], wsb[:, ec, c0:c0 + cn],
                            xT[:, ec, t4 * 512:(t4 + 1) * 512],
                            start=(ec == 0), stop=(ec == EC - 1))
                    nc.scalar.copy(dst[:, t4 * 512:(t4 + 1) * 512], pp[:cn, :])

            xtpool.__exit__(None, None, None)
            _gcm = tc.tile_pool(name="gum", bufs=4)
            gpool = _gcm.__enter__()
            _acm = tc.tile_pool(name="apg", bufs=2)
            apool = _acm.__enter__()
            _bcm = tc.tile_pool(name="idxb", bufs=2)
            bpool = _bcm.__enter__()

            # ---- Phase 3: v natural [128, NB, 192] in SBUF (via PE transposes)
            v_nat = bigpool.tile([128, NB, HPC * HD], F32, tag="v_nat")
            for tb in range(NB):
                pv = ptr.tile([128, 128], F32, tag="tr")
                nc.tensor.transpose(pv, vtA[:, tb * 128:(tb + 1) * 128], id_sb)
                pv2 = ptr.tile([128, 64], F32, tag="tr")
                nc.tensor.transpose(pv2, vtB[:, tb * 128:(tb + 1) * 128], id_sb[0:64, 0:64])
                nc.scalar.copy(v_nat[:, tb, 0:128], pv)
                nc.scalar.copy(v_nat[:, tb, 128:192], pv2)

            # ---- Phase 4: att + gumbel, argmax -> idx, per head
            yTA = bigpool.tile([128, T], F32, tag="yTA")   # heads 0,1
            yTB = bigpool.tile([64, T], F32, tag="yTB")    # head 2

            for h in range(HPC):
                if h < 2:
                    qt, kt, qo = qtA, ktA, 64 * h
                else:
                    qt, kt, qo = qtB, ktB, 0
                idxall = spool.tile([128, NB], F32, tag="idxall")
                for i in range(NB):
                    W = (i + 1) * 128
                    apg = apool.tile([128, T], F32, tag="apg")
                    for c4 in range(0, W, 512):
                        cw = min(512, W - c4)
                        pa = pmm.tile([128, 512], F32, tag="mm")
                        nc.tensor.matmul(
                            pa[:, :cw],
                            qt[qo:qo + 64, i * 128:(i + 1) * 128],
                            kt[qo:qo + 64, c4:c4 + cw],
                            start=True, stop=True)
                        gt = gpool.tile([128, 512], F32, tag="gum")
                        nc.sync.dma_start(
                            gt[:, :cw], gum[h, i * 128:(i + 1) * 128, c4:c4 + cw])
                        nc.vector.tensor_tensor(
                            apg[:, c4:c4 + cw], pa[:, :cw], gt[:, :cw],
                            mybir.AluOpType.add)
                    # causal mask on the diagonal block
                    nc.vector.tensor_tensor(
                        apg[:, i * 128:W], apg[:, i * 128:W], md_sb,
                        mybir.AluOpType.add)
                    mx8 = spool.tile([128, 8], F32, tag="mx8")
                    nc.vector.max(mx8, apg[:, :W])
                    ix8 = spool.tile([128, 8], U16, tag="ix8")
                    nc.vector.max_index(ix8, mx8, apg[:, :W])
                    nc.vector.tensor_copy(idxall[:, i:i + 1], ix8[:, 0:1])  # u16 -> f32 cast

                # idx column -> row layout: PE transpose, then broadcast down
                # partitions via ones-matmul; one-hot by iota compare; y = ohT.T @ v
                if debug:
                    nc.sync.dma_start(idx_dbg[h], idxall)
                pidxT = ptr.tile([16, 128], F32, tag="tr")
                nc.tensor.transpose(pidxT, idxall, id_sb)
                idxT = spool.tile([16, 128], F32, tag="idxT")
                nc.scalar.copy(idxT, pidxT)

                for I in range(NB // 4):          # superblocks of 4 q-blocks
                    pbc = pmm.tile([128, 512], F32, tag="mm")
                    for j in range(4):
                        ib = (4 * I + j) * 128
                        nc.tensor.matmul(
                            pbc[:, j * 128:(j + 1) * 128],
                            on_sb[:, ib:ib + 128], idxT,
                            start=True, stop=True)
                    idxb = bpool.tile([128, 512], F32, tag="idxb")
                    nc.scalar.copy(idxb, pbc)

                    pyt = pmm.tile([64, 512], F32, tag="yt")
                    nch = 4 * I + 4
                    for c in range(nch):
                        oh = bpool.tile([128, 512], F32, tag="oh")
                        nc.vector.tensor_scalar(
                            oh, idxb, io_sb[:, c:c + 1], None,
                            mybir.AluOpType.is_equal)
                        nc.tensor.matmul(
                            pyt, v_nat[:, c, h * HD:(h + 1) * HD], oh,
                            start=(c == 0), stop=(c == nch - 1))
                    if h < 2:
                        nc.scalar.copy(yTA[64 * h:64 * h + 64, I * 512:(I + 1) * 512], pyt)
                    else:
                        nc.scalar.copy(yTB[:, I * 512:(I + 1) * 512], pyt)

            if debug:
                nc.sync.dma_start(yta_dbg[:, :], yTA)
                nc.sync.dma_start(ytb_dbg[:, :], yTB)

            # ---- Phase 5: c_proj partial: out[t,o] = sum_c yT[c,t] * wp[c,o]
            wpA = bigpool.tile([128, C], F32, tag="wpA")
            wpB = bigpool.tile([64, C], F32, tag="wpB")
            nc.sync.dma_start(wpA, wp[0:128, :])
            nc.sync.dma_start(wpB, wp[128:192, :])
            for tb in range(NB):
                po = psopool.tile([128, C], F32, tag="proj")
                for fc, fw in ((0, 512), (512, 256)):
                    nc.tensor.matmul(
                        po[:, fc:fc + fw], yTA[:, tb * 128:(tb + 1) * 128],
                        wpA[:, fc:fc + fw], start=True, stop=False)
                    nc.tensor.matmul(
                        po[:, fc:fc + fw], yTB[:, tb * 128:(tb + 1) * 128],
                        wpB[:, fc:fc + fw], start=False, stop=True)
                ost = iopool.tile([128, C], F32, tag="ost")
                nc.scalar.copy(ost, po)
                nc.sync.dma_start(out[tb * 128:(tb + 1) * 128, :], ost)

            _bcm.__exit__(None, None, None)
            _acm.__exit__(None, None, None)
            _gcm.__exit__(None, None, None)

    nc.finalize()
    return nc


_NC_CACHE = {}


def kernel(x, w_attn, b_attn, w_proj, b_proj, gumbel, _trace=False):
    B, T_, C_ = x.shape
    H = 12
    assert (B, T_, C_) == (2, T, C)
    assert np.all(b_attn == 0.0), "kernel assumes zero attn bias"
    scale = np.float32(1.0 / np.sqrt(HD))

    if "nc" not in _NC_CACHE:
        _NC_CACHE["nc"] = build_program()
    nc = _NC_CACHE["nc"]

    ident = np.eye(128, dtype=np.float32)
    # mdiag[p, j] = 0 if j <= p else -1e9   (local q=p, k=j within diag block)
    jj = np.arange(128)
    mdiag = np.where(jj[None, :] <= jj[:, None], 0.0, NEG).astype(np.float32)
    iotas = (jj[:, None] + 128.0 * np.arange(NB)[None, :]).astype(np.float32)
    onesel = np.kron(np.eye(16, dtype=np.float32), np.ones((1, 128), np.float32))
    onesel = np.ascontiguousarray(onesel)  # [16, 2048]

    in_maps = []
    for core in range(8):
        b, h0 = core // 4, HPC * (core % 4)
        cq = slice(h0 * HD, (h0 + HPC) * HD)
        in_maps.append({
            "x": np.ascontiguousarray(x[b]),
            "wq": np.ascontiguousarray(w_attn[:, cq.start:cq.stop]) * scale,
            "wk": np.ascontiguousarray(w_attn[:, C + cq.start:C + cq.stop]),
            "wv": np.ascontiguousarray(w_attn[:, 2 * C + cq.start:2 * C + cq.stop]),
            "wp": np.ascontiguousarray(w_proj[cq, :]),
            "gum": np.ascontiguousarray(gumbel[b, h0:h0 + HPC]),
            "ident": ident,
            "mdiag": mdiag,
            "iotas": iotas,
            "onesel": onesel,
        })

    res = run_bass_kernel_spmd(nc, in_maps, core_ids=list(range(8)), trace=_trace)
    parts = [r["out"] for r in res.results]
    outp = np.empty((B, T, C), dtype=np.float32)
    for b in range(B):
        outp[b] = parts[4 * b] + parts[4 * b + 1] + parts[4 * b + 2] + parts[4 * b + 3]
        outp[b] += b_proj[None, :]
    if _trace:
        return outp, res
    return outp

